# revision 1
# baseline (speedup 1.0000x reference)
"""Trainium2 Bass kernel for nn_Block_3616362463321 (dense transformer block).

B=8, T=1024, C=1024, H=16, Dh=64. Data-parallel over batch: core b gets x[b].
Weights replicated to all 8 cores; no collectives.

Per-core layout strategy: activations live TRANSPOSED in SBUF ([C_part, T_free])
so every matmul consumes weights in natural [c_in, c_out] layout as lhsT and
activations as rhs, with no transposes inside the chain:
  - xn^T = LN1(x)^T           (stats via bn_stats on normal x tiles,
                               normalize in transposed space w/ broadcast rows,
                               split into independent t-halves so downstream
                               matmuls start on half 0 early)
  - Q^T, K^T = Wq/Wk^T chunks (lhsT=W[:, mchunk], rhs=xn^T)   [2 heads/chunk]
  - V natural = xn @ Wv       (lhsT=xn^T t-slice, rhs=Wv)  + ones column
  - S_h^T = K_h^T' @ Q_h^T    (K=64 contraction, 2 heads in row groups 0/64)
  - E = exp(S^T/8) with causal mask; blocks below the diagonal skipped
  - A_h^T|sums = [V_h|1]' @ E (M=65; fused softmax denominator in row 64)
  - attn^T = A^T / sums       (broadcast reciprocal via DRAM bounce)
  - y = attn^T' @ Wproj + x   (normal orientation; residual from reloaded x)
  - xn2^T = LN2(y)^T          (PE-transpose y, bn_stats on normal y)
  - h^T = relu(W1' @ xn2^T + b1)
  - out = h^T' @ W2 + y + b2  (normal orientation, DMA straight out)

All matmuls run float32r (TF32-like, 1 cycle/row at N>=256; measured rel err
~1.5e-4) with fp32 PSUM accumulation.  SBUF is managed as one arena pool whose
tag groups are reused across phase lifetimes; late-phase weights sit in groups
that die early so their DMAs prefetch during earlier compute.
"""
import sys

sys.path.insert(0, "/opt/trn_rl_repo")

from contextlib import ExitStack, nullcontext

import numpy as np

import concourse.bacc as bacc
import concourse.bass as bass
import concourse.mybir as mybir
import concourse.tile as tile
from concourse.bass_utils import run_bass_kernel_spmd
from concourse.masks import make_identity

P = 128
B, T, C, H = 8, 1024, 1024, 16
Dh = C // H            # 64
EPS = 1e-5
NF = 512               # matmul moving free dim (fp32 PSUM bank limit)
KC = C // P            # 8 c-chunks
TJ = T // P            # 8 t-chunks of 128
TN = T // NF           # 2 t-chunks of 512
F32 = mybir.dt.float32
F32R = mybir.dt.float32r
ALU = mybir.AluOpType
ACTF = mybir.ActivationFunctionType

N_CORES = 8

_CACHE = {}


def _bcast_row_ap(handle_ap, parts):
    """AP reading a [N]-shaped DRAM tensor broadcast across `parts` partitions."""
    return bass.AP(
        tensor=handle_ap.tensor,
        offset=handle_ap.offset,
        ap=[[0, parts], *handle_ap.ap],
    )


def build_nc(loop=1, hwloop=0, phases=7):
    nc = bacc.Bacc("TRN2", target_bir_lowering=False, debug=False)

    x_d = nc.dram_tensor("x", [T, C], F32, kind="ExternalInput")
    wq_d = nc.dram_tensor("wq", [C, C], F32R, kind="ExternalInput")   # [c, (h d)]
    wk_d = nc.dram_tensor("wk", [C, C], F32R, kind="ExternalInput")
    wv_d = nc.dram_tensor("wv", [C, C], F32R, kind="ExternalInput")
    wp_d = nc.dram_tensor("wp", [C, C], F32R, kind="ExternalInput")   # [c_in, c_out]
    w1_d = nc.dram_tensor("w1", [C, C], F32R, kind="ExternalInput")
    w2_d = nc.dram_tensor("w2", [C, C], F32R, kind="ExternalInput")
    bp_d = nc.dram_tensor("bp", [C], F32, kind="ExternalInput")
    b1_d = nc.dram_tensor("b1", [C], F32, kind="ExternalInput")
    b2_d = nc.dram_tensor("b2", [C], F32, kind="ExternalInput")
    g1_d = nc.dram_tensor("g1", [C], F32, kind="ExternalInput")
    be1_d = nc.dram_tensor("be1", [C], F32, kind="ExternalInput")
    g2_d = nc.dram_tensor("g2", [C], F32, kind="ExternalInput")
    be2_d = nc.dram_tensor("be2", [C], F32, kind="ExternalInput")
    out_d = nc.dram_tensor("out", [T, C], F32, kind="ExternalOutput")

    # causal mask for a diagonal [s,t] block of S^T: keep where s <= t
    mask_np = np.where(
        np.arange(P)[:, None] <= np.arange(P)[None, :], 0.0, -1e9
    ).astype(np.float32)
    mask_c = nc.inline_tensor(mask_np, name="mask_const")
    ident_c = nc.inline_tensor(np.eye(P, dtype=np.float32), name="ident_const")

    with tile.TileContext(nc) as tc, ExitStack() as ES:
        singles = ES.enter_context(tc.tile_pool(name="singles", bufs=1))
        dram = ES.enter_context(tc.tile_pool(name="drsc", bufs=1, space="DRAM"))

        ident = singles.tile([P, P], F32)
        nc.sync.dma_start(out=ident[:], in_=ident_c.ap())
        maskS = singles.tile([P, P], F32)
        nc.sync.dma_start(out=maskS[:], in_=mask_c.ap())
        zeros1 = singles.tile([P, 1], F32)
        nc.vector.memset(zeros1[:], 0.0)
        epsc = singles.tile([P, 1], F32)
        nc.vector.memset(epsc[:], EPS)
        ones1 = singles.tile([P, 1], F32)
        nc.vector.memset(ones1[:], 1.0)
        # per-chunk columns: [P, KC] with element (p, k) = vec[k*P + p].
        # Only g1/be1 are needed early; the rest load later so the startup
        # DMA window belongs to x.
        cols = {}

        def load_col(nm, hd):
            t_ = singles.tile([P, KC], F32, tag=f"col_{nm}", name=f"col_{nm}")
            nc.sync.dma_start(out=t_[:], in_=hd.ap().rearrange("(k p) -> p k", p=P))
            cols[nm] = t_

        load_col("g1", g1_d)
        load_col("be1", be1_d)
        bpb = singles.tile([P, C], F32)
        b2b = singles.tile([P, C], F32)

        # DRAM scratch for LN stat rows: one tile per t-half so each half's
        # broadcast only waits on its own 4 chunk writes
        ln_m = [[dram.tile([1, NF], F32, tag=f"lnm{i}_{h}", name=f"lnm{i}_{h}")
                 for h in range(TN)] for i in range(2)]
        ln_r = [[dram.tile([1, NF], F32, tag=f"lnr{i}_{h}", name=f"lnr{i}_{h}")
                 for h in range(TN)] for i in range(2)]

        # One arena pool; lifetime groups share tags so SBUF is reused:
        #   G0: xT -> QT -> y      G1: xnT -> attnT -> yT -> hT
        #   G2: wk -> V -> xn2T    G3: KT -> w1    G4: wq -> wv -> wp -> w2
        # (wp/w1/w2 sit in groups that die early so their DMAs prefetch)
        arena = ES.enter_context(tc.tile_pool(name="arena", bufs=1))
        xrp = ES.enter_context(tc.tile_pool(name="xrp", bufs=3))

        # Single PSUM pool for the whole kernel: 8 bank-sized tags, reused by
        # rotation. Phase-scoped PSUM pools would insert release->alloc
        # barriers at every phase boundary; tag reuse pipelines instead.
        psum = ES.enter_context(tc.tile_pool(name="psum", bufs=1, space="PSUM"))
        _si = [0]
        _ai = [0]
        _tags8 = [f"S{i}" for i in range(4)] + [f"A{i}" for i in range(4)]

        def stile(shape, nm="s", full=True):
            if full:
                tag = _tags8[_si[0] % 8]
            else:
                tag = f"S{_si[0] % 4}"
            t = psum.tile(list(shape), F32, tag=tag, name=f"{nm}{_si[0]}")
            _si[0] += 1
            return t

        def atile(shape, nm="a"):
            t = psum.tile(list(shape), F32, tag=f"A{_ai[0] % 4}",
                          name=f"{nm}{_ai[0]}")
            _ai[0] += 1
            return t

        def garr(g, nm, shape=(P, T), dtype=F32):
            return [arena.tile(list(shape), dtype, tag=f"G{g}_{i}",
                               name=f"{nm}{i}") for i in range(KC)]

        # bulk transfers alternate between the two HWDGE queues (SP + ACT):
        # measured 273 GB/s/core split vs 163 GB/s on one queue
        _dq = [0]

        def bulk_dma(out, in_):
            eng = nc.sync if _dq[0] % 2 == 0 else nc.scalar
            _dq[0] += 1
            eng.dma_start(out=out, in_=in_)

        def ln_stats_pass(src_loader, ln_m_d, ln_r_d, ph, dst_T=None):
            """For each 128-row chunk j of a [T, C] normal-layout tensor:
            produce the tile, bn_stats -> mean/rstd columns -> DRAM rows,
            and PE-transpose the tile into dst_T chunks (if given)."""
            with ExitStack() as S:
                rows = S.enter_context(tc.tile_pool(name=f"rows{ph}", bufs=4))
                stp = S.enter_context(tc.tile_pool(name=f"stp{ph}", bufs=4))
                for j in range(TJ):
                    xj = src_loader(rows, j)
                    st = stp.tile([P, 2, 6], F32, tag="st")
                    xr2 = xj[:].rearrange("p (g f) -> p g f", f=NF)
                    for g in range(2):
                        nc.vector.bn_stats(out=st[:, g, :], in_=xr2[:, g, :])
                    mv = stp.tile([P, 2], F32, tag="mv")
                    nc.vector.bn_aggr(out=mv[:], in_=st[:])
                    jh, jo = divmod(j, TJ // TN)
                    nc.gpsimd.dma_start(out=ln_m_d[jh][0, jo * P:(jo + 1) * P],
                                        in_=mv[:, 0:1])
                    nc.gpsimd.dma_start(out=ln_r_d[jh][0, jo * P:(jo + 1) * P],
                                        in_=mv[:, 1:2])
                    if dst_T is not None:
                        for k in range(KC):
                            pt = stile([P, P], "pt")
                            nc.tensor.transpose(pt[:], xj[:, k * P:(k + 1) * P],
                                                ident[:])
                            nc.any.tensor_copy(
                                out=dst_T[k][:, j * P:(j + 1) * P], in_=pt[:])

        def ln_normalize(src_T, dst_T, ln_m_d, ln_r_d, gcol, bcol, ph):
            """dst^T = g*(src^T - mean)*rstd + beta, per t-half."""
            with ExitStack() as S:
                bc = S.enter_context(tc.tile_pool(name=f"bc{ph}", bufs=1))
                for tn in range(TN):
                    tsl = slice(tn * NF, (tn + 1) * NF)
                    mb = bc.tile([P, NF], F32, tag=f"mb{tn}")
                    rb = bc.tile([P, NF], F32, tag=f"rb{tn}")
                    nc.gpsimd.dma_start(
                        out=mb[:], in_=ln_m_d[tn][0:1, :].to_broadcast([P, NF]))
                    nc.gpsimd.dma_start(
                        out=rb[:], in_=ln_r_d[tn][0:1, :].to_broadcast([P, NF]))
                    nc.scalar.activation(out=rb[:], in_=rb[:], func=ACTF.Sqrt,
                                         bias=epsc[:], scale=1.0)
                    nc.vector.reciprocal(rb[:], rb[:])
                    for k in range(KC):
                        nc.vector.tensor_tensor(dst_T[k][:, tsl],
                                                src_T[k][:, tsl], mb[:],
                                                ALU.subtract)
                        nc.vector.tensor_tensor(dst_T[k][:, tsl],
                                                dst_T[k][:, tsl], rb[:],
                                                ALU.mult)
                        nc.vector.tensor_scalar(
                            out=dst_T[k][:, tsl], in0=dst_T[k][:, tsl],
                            scalar1=gcol[:, k:k + 1], scalar2=bcol[:, k:k + 1],
                            op0=ALU.mult, op1=ALU.add)

        def load_x(rows, j):
            xj = rows.tile([P, C], F32, tag="xrow")
            bulk_dma(out=xj[:], in_=x_d[j * P:(j + 1) * P, :])
            return xj

        with (tc.For_i(0, hwloop, 1) if hwloop else nullcontext()):
            for _it in range(loop):
                # ---------- Phase 0+1: load x, stats, transpose, LN1 ----------
                xT = garr(0, "xT")
                ln_stats_pass(load_x, ln_m[0], ln_r[0], 0, dst_T=xT)
                xnT = garr(1, "xnT", dtype=F32R)
                ln_normalize(xT, xnT, ln_m[0], ln_r[0],
                             cols["g1"], cols["be1"], 0)

                if phases >= 2:
                    # ---------------- Phase 2: QKV ----------------
                    wq_sb = garr(4, "wq", (P, C), F32R)
                    wk_sb = garr(2, "wk", (P, C), F32R)
                    QT = garr(0, "QT", (P, T), F32R)
                    KT = garr(3, "KT", (P, T), F32R)
                    with ExitStack() as S:
                        for k in range(KC):
                            bulk_dma(out=wq_sb[k][:],
                                     in_=wq_d[k * P:(k + 1) * P, :])
                            bulk_dma(out=wk_sb[k][:],
                                     in_=wk_d[k * P:(k + 1) * P, :])
                        for tn in range(TN):
                            tsl = slice(tn * NF, (tn + 1) * NF)
                            for m in range(KC):
                                pq = stile([P, NF], "pq")
                                for k in range(KC):
                                    nc.tensor.matmul(
                                        pq[:], lhsT=wq_sb[k][:, m * P:(m + 1) * P],
                                        rhs=xnT[k][:, tsl],
                                        start=(k == 0), stop=(k == KC - 1))
                                nc.any.tensor_copy(out=QT[m][:, tsl], in_=pq[:])
                                pk = stile([P, NF], "pk")
                                for k in range(KC):
                                    nc.tensor.matmul(
                                        pk[:], lhsT=wk_sb[k][:, m * P:(m + 1) * P],
                                        rhs=xnT[k][:, tsl],
                                        start=(k == 0), stop=(k == KC - 1))
                                nc.any.tensor_copy(out=KT[m][:, tsl], in_=pk[:])

                    wv_sb = garr(4, "wv", (P, C), F32R)
                    V = garr(2, "V", (P, H, Dh + 1), F32R)
                    with ExitStack() as S:
                        for k in range(KC):
                            bulk_dma(out=wv_sb[k][:],
                                     in_=wv_d[k * P:(k + 1) * P, :])
                        for j in range(TJ):
                            nc.vector.tensor_copy(
                                out=V[j][:, :, Dh:Dh + 1],
                                in_=ones1[:, None, 0:1].to_broadcast([P, H, 1]))
                            for hn in range(TN):   # head groups of 8
                                pv = stile([P, NF], "pv")
                                for k in range(KC):
                                    nc.tensor.matmul(
                                        pv[:], lhsT=xnT[k][:, j * P:(j + 1) * P],
                                        rhs=wv_sb[k][:, hn * NF:(hn + 1) * NF],
                                        start=(k == 0), stop=(k == KC - 1))
                                nc.any.tensor_copy(
                                    out=V[j][:, hn * 8:(hn + 1) * 8, 0:Dh],
                                    in_=pv[:].rearrange("p (h d) -> p h d", d=Dh))

                if phases >= 3:
                    # ---------------- Phase 3: attention ----------------
                    attnT = garr(1, "attnT", (P, T), F32R)
                    with ExitStack() as S:
                        ep = S.enter_context(tc.tile_pool(name="ep", bufs=3))
                        rp = S.enter_context(tc.tile_pool(name="rp", bufs=2))
                        for m in range(KC):
                            h0, h1 = 2 * m, 2 * m + 1
                            for tn in range(TN):
                                tsl = slice(tn * NF, (tn + 1) * NF)
                                pa0 = atile([65, NF], "pa0")
                                pa1 = atile([65, NF], "pa1")
                                i_hi = 4 * (tn + 1)
                                for i in range(i_hi):
                                    diag = i - 4 * tn
                                    ssl = slice(i * P, (i + 1) * P)
                                    ps0 = stile([P, NF], "ps0", full=False)
                                    ps1 = stile([P, NF], "ps1", full=False)
                                    nc.tensor.matmul(ps0[:], lhsT=KT[m][0:64, ssl],
                                                     rhs=QT[m][0:64, tsl],
                                                     start=True, stop=True)
                                    nc.tensor.matmul(ps1[:], lhsT=KT[m][64:128, ssl],
                                                     rhs=QT[m][64:128, tsl],
                                                     start=True, stop=True)
                                    E0 = ep.tile([P, NF], F32R, tag="E0")
                                    E1 = ep.tile([P, NF], F32R, tag="E1")
                                    if diag >= 0:
                                        dsl = slice(diag * P, (diag + 1) * P)
                                        nc.vector.tensor_tensor(
                                            ps0[:, dsl], ps0[:, dsl], maskS[:],
                                            ALU.add)
                                        nc.vector.tensor_tensor(
                                            ps1[:, dsl], ps1[:, dsl], maskS[:],
                                            ALU.add)
                                    d0_raw = max(diag, 0) * P
                                    d0 = min(d0_raw, NF - 256)
                                    esl = slice(d0, NF)
                                    nc.scalar.activation(
                                        out=E0[:, esl], in_=ps0[:, esl],
                                        func=ACTF.Exp, scale=Dh ** -0.5)
                                    nc.scalar.activation(
                                        out=E1[:, esl], in_=ps1[:, esl],
                                        func=ACTF.Exp, scale=Dh ** -0.5)
                                    if d0 < d0_raw:
                                        zsl = slice(d0, d0_raw)
                                        zw = d0_raw - d0
                                        nc.vector.tensor_copy(
                                            out=E0[:, zsl],
                                            in_=zeros1[:, 0:1].to_broadcast([P, zw]))
                                        nc.vector.tensor_copy(
                                            out=E1[:, zsl],
                                            in_=zeros1[:, 0:1].to_broadcast([P, zw]))
                                    psl = slice(d0, NF)
                                    nc.tensor.matmul(
                                        pa0[:, psl], lhsT=V[i][:, h0, :],
                                        rhs=E0[:, psl],
                                        start=(i == 0), stop=(i == i_hi - 1))
                                    nc.tensor.matmul(
                                        pa1[:, psl], lhsT=V[i][:, h1, :],
                                        rhs=E1[:, psl],
                                        start=(i == 0), stop=(i == i_hi - 1))
                                # normalize by row 64 (softmax denominator)
                                r0 = rp.tile([1, NF], F32, tag="r0")
                                r1 = rp.tile([1, NF], F32, tag="r1")
                                nc.vector.reciprocal(r0[:], pa0[64:65, :])
                                nc.vector.reciprocal(r1[:], pa1[64:65, :])
                                drs = dram.tile([2, NF], F32, tag="sums")
                                nc.sync.dma_start(out=drs[0:1, :], in_=r0[:])
                                nc.sync.dma_start(out=drs[1:2, :], in_=r1[:])
                                rb0 = rp.tile([64, NF], F32, tag="rb0")
                                rb1 = rp.tile([64, NF], F32, tag="rb1")
                                nc.sync.dma_start(
                                    out=rb0[:], in_=drs[0:1, :].to_broadcast([64, NF]))
                                nc.sync.dma_start(
                                    out=rb1[:], in_=drs[1:2, :].to_broadcast([64, NF]))
                                nc.vector.tensor_tensor(attnT[m][0:64, tsl],
                                                        pa0[0:64, :], rb0[:],
                                                        ALU.mult)
                                tmp1 = rp.tile([64, NF], F32R, tag="tmp1")
                                nc.vector.tensor_tensor(tmp1[:], pa1[0:64, :],
                                                        rb1[:], ALU.mult)
                                nc.sync.dma_start(out=attnT[m][64:128, tsl],
                                              in_=tmp1[:])

                if phases >= 4:
                    # ---------- Phase 4: proj + residual -> y (normal) ----------
                    if _it == 0:
                        bulk_dma(out=bpb[:], in_=_bcast_row_ap(bp_d.ap(), P))
                        bulk_dma(out=b2b[:], in_=_bcast_row_ap(b2_d.ap(), P))
                        load_col("g2", g2_d)
                        load_col("be2", be2_d)
                        load_col("b1", b1_d)
                    wp_sb = garr(4, "wp", (P, C), F32R)
                    y_n = garr(0, "y", (P, C), F32)
                    with ExitStack() as S:
                        xrp = S.enter_context(tc.tile_pool(name="xrp", bufs=3))
                        for k in range(KC):
                            bulk_dma(out=wp_sb[k][:],
                                     in_=wp_d[k * P:(k + 1) * P, :])
                        for j in range(TJ):
                            xr = xrp.tile([P, C], F32, tag="xr")
                            bulk_dma(out=xr[:], in_=x_d[j * P:(j + 1) * P, :])
                            for nn in range(TN):
                                csl = slice(nn * NF, (nn + 1) * NF)
                                pp = stile([P, NF], "pp")
                                for k in range(KC):
                                    nc.tensor.matmul(
                                        pp[:], lhsT=attnT[k][:, j * P:(j + 1) * P],
                                        rhs=wp_sb[k][:, csl],
                                        start=(k == 0), stop=(k == KC - 1))
                                nc.vector.tensor_tensor(y_n[j][:, csl], pp[:],
                                                        xr[:, csl], ALU.add)
                                nc.vector.tensor_tensor(y_n[j][:, csl],
                                                        y_n[j][:, csl],
                                                        bpb[:, csl], ALU.add)

                if phases >= 5:
                    # ---------------- Phase 5: LN2 ----------------
                    yT = garr(1, "yT", (P, T), F32)

                    def load_y(rows, j):
                        return y_n[j]

                    ln_stats_pass(load_y, ln_m[1], ln_r[1], 1, dst_T=yT)
                    xn2T = garr(2, "xn2T", (P, T), F32R)
                    ln_normalize(yT, xn2T, ln_m[1], ln_r[1],
                                 cols["g2"], cols["be2"], 1)

                if phases >= 6:
                    # ---------------- Phase 6: MLP fc1 + relu ----------------
                    w1_sb = garr(3, "w1", (P, C), F32R)
                    hT = garr(1, "hT", (P, T), F32R)
                    with ExitStack() as S:
                        for k in range(KC):
                            bulk_dma(out=w1_sb[k][:],
                                     in_=w1_d[k * P:(k + 1) * P, :])
                        for tn in range(TN):
                            tsl = slice(tn * NF, (tn + 1) * NF)
                            for m in range(KC):
                                ph = stile([P, NF], "ph")
                                for k in range(KC):
                                    nc.tensor.matmul(
                                        ph[:], lhsT=w1_sb[k][:, m * P:(m + 1) * P],
                                        rhs=xn2T[k][:, tsl],
                                        start=(k == 0), stop=(k == KC - 1))
                                nc.scalar.activation(out=hT[m][:, tsl], in_=ph[:],
                                                     func=ACTF.Relu,
                                                     bias=cols["b1"][:, m:m + 1],
                                                     scale=1.0)

                if phases >= 7:
                    # ---------- Phase 7: MLP fc2 + residual -> out ----------
                    w2_sb = garr(4, "w2", (P, C), F32R)
                    with ExitStack() as S:
                        otp = S.enter_context(tc.tile_pool(name="otp", bufs=3))
                        for k in range(KC):
                            bulk_dma(out=w2_sb[k][:],
                                     in_=w2_d[k * P:(k + 1) * P, :])
                        for j in range(TJ):
                            for nn in range(TN):
                                csl = slice(nn * NF, (nn + 1) * NF)
                                po = stile([P, NF], "po")
                                for k in range(KC):
                                    nc.tensor.matmul(
                                        po[:], lhsT=hT[k][:, j * P:(j + 1) * P],
                                        rhs=w2_sb[k][:, csl],
                                        start=(k == 0), stop=(k == KC - 1))
                                ot = otp.tile([P, NF], F32, tag="ot")
                                nc.vector.tensor_tensor(ot[:], po[:],
                                                        y_n[j][:, csl], ALU.add)
                                nc.vector.tensor_tensor(ot[:], ot[:], b2b[:, csl],
                                                        ALU.add)
                                bulk_dma(out=out_d[j * P:(j + 1) * P, csl],
                                         in_=ot[:])

    nc.compile()
    return nc


def _prep_inputs(inputs):
    """Host-side weight repacking; returns per-core in_maps."""
    f = np.float32
    x = np.ascontiguousarray(np.asarray(inputs["x"], dtype=f))        # [B, T, C]
    wq = np.ascontiguousarray(
        np.asarray(inputs["Wq"], dtype=f).transpose(1, 0, 2).reshape(C, C))
    wk = np.ascontiguousarray(
        np.asarray(inputs["Wk"], dtype=f).transpose(1, 0, 2).reshape(C, C))
    wv = np.ascontiguousarray(
        np.asarray(inputs["Wv"], dtype=f).transpose(1, 0, 2).reshape(C, C))
    common = {
        "wq": wq, "wk": wk, "wv": wv,
        "wp": np.ascontiguousarray(np.asarray(inputs["Wproj"], dtype=f)),
        "w1": np.ascontiguousarray(np.asarray(inputs["W1"], dtype=f)),
        "w2": np.ascontiguousarray(np.asarray(inputs["W2"], dtype=f)),
        "bp": np.asarray(inputs["bproj"], dtype=f),
        "b1": np.asarray(inputs["b1"], dtype=f),
        "b2": np.asarray(inputs["b2"], dtype=f),
        "g1": np.asarray(inputs["g1"], dtype=f),
        "be1": np.asarray(inputs["beta1"], dtype=f),
        "g2": np.asarray(inputs["g2"], dtype=f),
        "be2": np.asarray(inputs["beta2"], dtype=f),
    }
    return [{"x": x[b], **common} for b in range(N_CORES)]


def kernel(**inputs) -> np.ndarray:
    if "nc" not in _CACHE:
        _CACHE["nc"] = build_nc()
    nc = _CACHE["nc"]
    in_maps = _prep_inputs(inputs)
    res = run_bass_kernel_spmd(nc, in_maps, list(range(N_CORES)))
    out = np.stack([res.results[b]["out"] for b in range(N_CORES)], axis=0)
    return out.astype(np.float32)


if __name__ == "__main__":
    rng = np.random.default_rng(0)
    demo = {
        "x": rng.standard_normal((B, T, C), dtype=np.float32),
        "Wq": rng.standard_normal((H, C, Dh), dtype=np.float32) * 0.02,
        "Wk": rng.standard_normal((H, C, Dh), dtype=np.float32) * 0.02,
        "Wv": rng.standard_normal((H, C, Dh), dtype=np.float32) * 0.02,
        "Wproj": rng.standard_normal((C, C), dtype=np.float32) * 0.02,
        "bproj": np.zeros(C, np.float32),
        "W1": rng.standard_normal((C, C), dtype=np.float32) * 0.02,
        "b1": np.zeros(C, np.float32),
        "W2": rng.standard_normal((C, C), dtype=np.float32) * 0.02,
        "b2": np.zeros(C, np.float32),
        "g1": np.ones(C, np.float32),
        "beta1": np.zeros(C, np.float32),
        "g2": np.ones(C, np.float32),
        "beta2": np.zeros(C, np.float32),
    }
    y = kernel(**demo)
    print("out", y.shape, y.dtype, float(np.abs(y).max()))



# revision 20
# speedup vs baseline: 1.0073x; 1.0073x over previous
"""Trainium2 Bass kernel for nn_Block_3616362463321 (dense transformer block).

B=8, T=1024, C=1024, H=16, Dh=64. Data-parallel over batch: core b gets x[b].
Weights replicated to all 8 cores; no collectives.

v2 design (vs baseline):
  - All six weight GEMMs (QKV / proj / fc1 / fc2) run fp8e4m3 with
    perf_mode=DoubleRow: K=256 contraction per pass, 0.5 cycles/row.
    Weights are host-quantized (x1024, clip +-240) into an interleaved
    [q, p, islot, m] layout; activations are quantized on the fly into
    "mega" SBUF tiles [128, 8*1024] fp8 whose (k=c-chunk, t) layout serves
    both the DoubleRow moving-operand view [p, 2, t] and the stationary
    view [p, 2, 128].
  - LayerNorm statistics AND normalization happen in natural [t, c] layout
    (per-partition mean/rstd -> one tensor_scalar), with gamma folded into
    the weights host-side and beta folded into per-output bias columns.
    No DRAM stat bounces.  Normalized activations are PE-transposed as fp8
    (1 cycle/row), 4 blocks batched per PSUM bank.
  - Attention keeps the baseline S^T orientation (fp8 Q/K, bf16 E,
    fp8 V with fused 0.125-ones column for softmax denominators), with:
      * S row-tile pairs (K=64 at rows 0/64) writing one 2-bank PSUM pair
        tile -> a single paired exp per (m, tn, i) on ACT,
      * exact causal trims everywhere (bf16/fp8 matmuls have no N>=256
        restriction),
      * causal masking as a post-exp bf16 triangle multiply (2x DVE mode),
      * reciprocal_approx_fast for denominators + SBUF->SBUF broadcast DMA,
  - Residual/dequant fused into single scalar_tensor_tensor ops; SBUF-only
    elementwise work (x+bias rows) offloaded to the idle GpSimd engine.
  - Bulk DMA spread over the SP / Pool / ACT hardware queues.
"""
import sys

sys.path.insert(0, "/opt/trn_rl_repo")

from contextlib import ExitStack, nullcontext

import numpy as np
import ml_dtypes

import concourse.bacc as bacc
import concourse.bass as bass
import concourse.mybir as mybir
import concourse.tile as tile
from concourse.bass_utils import run_bass_kernel_spmd

P = 128
B, T, C, H = 8, 1024, 1024, 16
Dh = C // H            # 64
EPS = 1e-5
NF = 512               # matmul moving free dim (fp32 PSUM bank limit)
KC = C // P            # 8 c-chunks of 128
QC = C // 256          # 4 c-chunks of 256 (DoubleRow)
TJ = T // P            # 8 t-chunks of 128
TN = T // NF           # 2 t-chunks of 512
F32 = mybir.dt.float32
F32R = mybir.dt.float32r
BF16 = mybir.dt.bfloat16
F8 = mybir.dt.float8e4
ALU = mybir.AluOpType
ACTF = mybir.ActivationFunctionType
DR = mybir.MatmulPerfMode.DoubleRow

WS = 1024.0            # host weight upscale (fp8 range use)
QS = 8.0               # Q/K storage scale
SEXP = (Dh ** -0.5) / (QS * QS)   # exp scale absorbing Q/K storage scales
VIS = 1.0              # V ones-column value (bf16 attnT: true scale)
HS = 8.0               # hT storage scale

N_CORES = 8

_CACHE = {}

F8NP = ml_dtypes.float8_e4m3
BF16NP = ml_dtypes.bfloat16


def _bcast_row_ap(handle_ap, parts):
    """AP reading a [N]-shaped DRAM tensor broadcast across `parts` partitions."""
    return bass.AP(
        tensor=handle_ap.tensor,
        offset=handle_ap.offset,
        ap=[[0, parts], *handle_ap.ap],
    )


def build_nc(loop=1, hwloop=0, phases=7):
    nc = bacc.Bacc("TRN2", target_bir_lowering=False, debug=False)

    x_d = nc.dram_tensor("x", [T, C], F32, kind="ExternalInput")
    w_d = {}
    for nm in ("wq", "wk", "wv", "w1"):
        w_d[nm] = nc.dram_tensor(nm, [QC * P, 2 * C], F8, kind="ExternalInput")
    wp_d = nc.dram_tensor("wp", [C, C], BF16, kind="ExternalInput")
    w2_d = nc.dram_tensor("w2", [C, C], BF16, kind="ExternalInput")
    colq_d = nc.dram_tensor("colq", [C], F32, kind="ExternalInput")
    colk_d = nc.dram_tensor("colk", [C], F32, kind="ExternalInput")
    b1c_d = nc.dram_tensor("b1c", [C], F32, kind="ExternalInput")
    bvr_d = nc.dram_tensor("bvr", [C], BF16, kind="ExternalInput")
    bp_d = nc.dram_tensor("bp", [C], F32, kind="ExternalInput")
    b2_d = nc.dram_tensor("b2", [C], F32, kind="ExternalInput")
    out_d = nc.dram_tensor("out", [T, C], F32, kind="ExternalOutput")

    identb_c = nc.inline_tensor(np.eye(P).astype(BF16NP), name="identb_c")
    # post-exp causal keep-mask for a diagonal [s,t] block: keep where s <= t
    tri_np = (np.arange(P)[:, None] <= np.arange(P)[None, :]).astype(BF16NP)
    tri_c = nc.inline_tensor(tri_np, name="tri_c")

    with tile.TileContext(nc) as tc, ExitStack() as ES:
        singles = ES.enter_context(tc.tile_pool(name="singles", bufs=1))
        dram = ES.enter_context(tc.tile_pool(name="drsc", bufs=1, space="DRAM"))

        identb = singles.tile([P, P], BF16)
        nc.sync.dma_start(out=identb[:], in_=identb_c.ap())
        tri01 = singles.tile([P, P], BF16)
        nc.sync.dma_start(out=tri01[:], in_=tri_c.ap())
        epsc = singles.tile([P, 1], F32)
        nc.vector.memset(epsc[:], EPS)
        onesb = singles.tile([1, P], BF16)
        nc.vector.memset(onesb[:], 1.0)

        # bias columns [P, KC]: element (p, k) = vec[k*P + p]
        cols = {}

        def load_col(nm, hd):
            t_ = singles.tile([P, KC], F32, tag=f"col_{nm}", name=f"col_{nm}")
            nc.sync.dma_start(out=t_[:], in_=hd.ap().rearrange("(k p) -> p k", p=P))
            cols[nm] = t_

        load_col("q", colq_d)
        load_col("k", colk_d)
        load_col("b1", b1c_d)
        bvr = singles.tile([1, C], BF16)
        nc.sync.dma_start(out=bvr[:], in_=bvr_d.ap()[None, :])
        bpb = singles.tile([P, C], F32)
        nc.scalar.dma_start(out=bpb[:], in_=_bcast_row_ap(bp_d.ap(), P))
        b2b = singles.tile([P, C], F32)
        nc.scalar.dma_start(out=b2b[:], in_=_bcast_row_ap(b2_d.ap(), P))

        # ---- SBUF arena ----
        arena = ES.enter_context(tc.tile_pool(name="arena", bufs=1))

        def mega(tag):
            return arena.tile([P, KC * T], F8, tag=tag, name=tag)

        # weight tiles: [P, 2, C] fp8 per 256-chunk
        def wtiles(nm, share=None):
            tg = share or nm
            return [arena.tile([P, 2, C], F8, tag=f"{tg}_{q}", name=f"{nm}{q}")
                    for q in range(QC)]

        # ---- PSUM pool: declare SP pair tags first (2 banks each), then PA ----
        psum = ES.enter_context(tc.tile_pool(name="psum", bufs=1, space="PSUM"))
        _sp = [0]
        _pa = [0]

        def sptile(shape, dtype, nm="sp"):
            t = psum.tile(list(shape), dtype, tag=f"SP{_sp[0] % 2}",
                          name=f"{nm}{_sp[0]}")
            _sp[0] += 1
            return t

        def patile(shape=(P, NF), nm="pa"):
            t = psum.tile(list(shape), F32, tag=f"PA{_pa[0] % 4}",
                          name=f"{nm}{_pa[0]}")
            _pa[0] += 1
            return t

        # force tag declaration order: SP0, SP1 as [P, 2, NF] f32 (2 banks each)
        _ = psum.tile([P, 2, NF], F32, tag="SP0", name="spdecl0")
        _ = psum.tile([P, 2, NF], F32, tag="SP1", name="spdecl1")

        _dq = [0]
        _dqe = None

        def bulk_dma(out, in_):
            eng = _dqe[_dq[0] % len(_dqe)]
            _dq[0] += 1
            eng.dma_start(out=out, in_=in_)

        _dqe = [nc.sync, nc.gpsimd, nc.scalar]

        def ln_pass(src_tiles_or_loader, xn_tag, dst_mega, ph, out_rows_dtype=F8):
            """Natural-layout LN: per 128-row chunk j, bn_stats -> mean/rstd
            columns -> one tensor_scalar into an fp8 row tile -> PE-transpose
            (fp8, 4 blocks per PSUM batch) into dst_mega [(k t)] layout."""
            with ExitStack() as S:
                stp = S.enter_context(tc.tile_pool(name=f"stp{ph}", bufs=4))
                xnp = S.enter_context(tc.tile_pool(name=f"xnp{ph}", bufs=3))
                dv = dst_mega[:].rearrange("p (k t) -> p k t", k=KC)
                for j in range(TJ):
                    xj = src_tiles_or_loader(j)
                    st = stp.tile([P, 2, 6], F32, tag="st")
                    xr2 = xj[:].rearrange("p (g f) -> p g f", f=NF)
                    for g in range(2):
                        nc.vector.bn_stats(out=st[:, g, :], in_=xr2[:, g, :])
                    mv = stp.tile([P, 2], F32, tag="mv")
                    nc.vector.bn_aggr(out=mv[:], in_=st[:])
                    srt = stp.tile([P, 1], F32, tag="srt")
                    nc.scalar.activation(out=srt[:], in_=mv[:, 1:2],
                                         func=ACTF.Sqrt, bias=epsc[:], scale=1.0)
                    rc = stp.tile([P, 1], F32, tag="rc")
                    nc.vector.reciprocal(rc[:], srt[:])
                    xnr = xnp.tile([P, C], BF16, tag="xnr")
                    nc.vector.tensor_scalar(
                        out=xnr[:], in0=xj[:], scalar1=mv[:, 0:1], scalar2=rc[:],
                        op0=ALU.subtract, op1=ALU.mult)
                    for kb in range(2):
                        pt = sptile([P, 4 * P], BF16, "pt")
                        for k4 in range(4):
                            k = kb * 4 + k4
                            nc.tensor.transpose(pt[:, k4 * P:(k4 + 1) * P],
                                                xnr[:, k * P:(k + 1) * P],
                                                identb[:])
                        dst = dv[:, kb * 4:(kb + 1) * 4, j * P:(j + 1) * P]
                        src = pt[:].rearrange("p (a b) -> p a b", a=4)
                        if kb == 0:
                            nc.vector.tensor_copy(out=dst, in_=src)
                        else:
                            nc.scalar.activation(out=dst, in_=src,
                                                 func=ACTF.Copy)

        xrp = ES.enter_context(tc.tile_pool(name="xrp", bufs=3))

        def load_x(j):
            xj = xrp.tile([P, C], F32, tag="xrow")
            bulk_dma(out=xj[:], in_=x_d[j * P:(j + 1) * P, :])
            return xj

        with (tc.For_i(0, hwloop, 1) if hwloop else nullcontext()):
            for _it in range(loop):
                # ---------- weights: issue all DMAs up front (prefetch) ----------
                wq_sb = wtiles("wq")
                wk_sb = wtiles("wk")
                wv_sb = wtiles("wv")
                # w1/w2 reuse wv/wq slots (dead after QKV); DMAs self-order
                w1_sb = wtiles("w1", share="wv")
                w2_sb = [arena.tile([P, C], BF16,
                                    tag=(f"wq_{k}" if k < QC else f"wk_{k - QC}"),
                                    name=f"w2{k}") for k in range(KC)]
                for q in range(QC):
                    for nm, tl in (("wq", wq_sb), ("wk", wk_sb), ("wv", wv_sb)):
                        bulk_dma(out=tl[q][:], in_=w_d[nm][q * P:(q + 1) * P, :])
                # proj weights: plain bf16 [P, C] per c'-chunk
                wp_sb = [arena.tile([P, C], BF16, tag=f"wp_{k}", name=f"wp{k}")
                         for k in range(KC)]
                for k in range(KC):
                    bulk_dma(out=wp_sb[k][:], in_=wp_d[k * P:(k + 1) * P, :])

                # ---------- Phase 0: LN1 (stats + normalize + transpose) ----------
                xn8 = mega("XN1")
                ln_pass(load_x, "xn", xn8, 0)
                xnv = xn8[:].rearrange("p (k t) -> p k t", k=KC)

                if phases >= 2:
                    # ---------------- Phase 1: QKV (fp8 DoubleRow) ----------------
                    QT = [arena.tile([P, T], F8, tag=f"QT_{m}", name=f"QT{m}")
                          for m in range(KC)]
                    KTt = [arena.tile([P, T], F8, tag=f"KT_{m}", name=f"KT{m}")
                           for m in range(KC)]
                    for tn in range(TN):
                        tsl = slice(tn * NF, (tn + 1) * NF)
                        for m in range(KC):
                            pq = patile(nm="pq")
                            for q in range(QC):
                                nc.tensor.matmul(
                                    pq[:], lhsT=wq_sb[q][:, :, m * P:(m + 1) * P],
                                    rhs=xnv[:, 2 * q:2 * q + 2, tsl],
                                    start=(q == 0), stop=(q == QC - 1),
                                    perf_mode=DR)
                            nc.vector.tensor_scalar(
                                out=QT[m][:, tsl], in0=pq[:], scalar1=QS / WS,
                                scalar2=cols["q"][:, m:m + 1],
                                op0=ALU.mult, op1=ALU.add)
                            pk = patile(nm="pk")
                            for q in range(QC):
                                nc.tensor.matmul(
                                    pk[:], lhsT=wk_sb[q][:, :, m * P:(m + 1) * P],
                                    rhs=xnv[:, 2 * q:2 * q + 2, tsl],
                                    start=(q == 0), stop=(q == QC - 1),
                                    perf_mode=DR)
                            nc.scalar.activation(
                                out=KTt[m][:, tsl], in_=pk[:], func=ACTF.Identity,
                                scale=QS / WS, bias=cols["k"][:, m:m + 1])

                    V = [arena.tile([P, H, Dh + 1], BF16, tag=f"V_{j}",
                                    name=f"V{j}") for j in range(TJ)]
                    for j in range(TJ):
                        nc.gpsimd.memset(V[j][:, :, Dh:Dh + 1], VIS)
                        for hn in range(TN):
                            hsl = slice(hn * NF, (hn + 1) * NF)
                            pv = patile(nm="pv")
                            nc.tensor.matmul(pv[:], lhsT=onesb[0:1, :],
                                             rhs=bvr[0:1, hsl],
                                             start=True, stop=False)
                            for q in range(QC):
                                nc.tensor.matmul(
                                    pv[:], lhsT=xnv[:, 2 * q:2 * q + 2,
                                                    j * P:(j + 1) * P],
                                    rhs=wv_sb[q][:, :, hsl],
                                    start=False, stop=(q == QC - 1),
                                    perf_mode=DR)
                            nc.scalar.activation(
                                out=V[j][:, hn * 8:(hn + 1) * 8, 0:Dh],
                                in_=pv[:].rearrange("p (h d) -> p h d", d=Dh),
                                func=ACTF.Identity, scale=1.0 / WS)
                    # late weights into the now-free wv/wq/wk slots
                    for q in range(QC):
                        bulk_dma(out=w1_sb[q][:],
                                 in_=w_d["w1"][q * P:(q + 1) * P, :])
                    for k in range(KC):
                        bulk_dma(out=w2_sb[k][:],
                                 in_=w2_d[k * P:(k + 1) * P, :])

                attnT = arena.tile([P, KC * T], BF16, tag="ATT", name="attnT")
                atv = attnT[:].rearrange("p (m t) -> p m t", m=KC)
                if phases >= 3:
                    # ---------------- Phase 2: attention ----------------
                    with ExitStack() as S:
                        ep = S.enter_context(tc.tile_pool(name="ep", bufs=3))
                        rp = S.enter_context(tc.tile_pool(name="rp", bufs=2))
                        bp_ = S.enter_context(tc.tile_pool(name="bp", bufs=2))
                        tp1 = S.enter_context(tc.tile_pool(name="tp1", bufs=2))
                        for tn in range(TN):
                            tsl = slice(tn * NF, (tn + 1) * NF)
                            i_hi = 4 * (tn + 1)
                            for m in range(KC):
                                h0, h1 = 2 * m, 2 * m + 1
                                pa0 = patile((Dh + 1, NF), "pa0")
                                pa1 = patile((Dh + 1, NF), "pa1")
                                for i in range(i_hi):
                                    diag = i - 4 * tn
                                    d0 = max(diag, 0) * P
                                    esl = slice(d0, NF)
                                    qsl = slice(tn * NF + d0, (tn + 1) * NF)
                                    ssl = slice(i * P, (i + 1) * P)
                                    sp2 = sptile([P, 2, NF], F32, "s")
                                    nc.tensor.matmul(
                                        sp2[:, 0, esl], lhsT=KTt[m][0:64, ssl],
                                        rhs=QT[m][0:64, qsl],
                                        start=True, stop=True)
                                    nc.tensor.matmul(
                                        sp2[:, 1, esl], lhsT=KTt[m][64:128, ssl],
                                        rhs=QT[m][64:128, qsl],
                                        start=True, stop=True)
                                    Et = ep.tile([P, 2, NF], BF16, tag="E")
                                    nc.scalar.activation(
                                        out=Et[:, :, esl], in_=sp2[:, :, esl],
                                        func=ACTF.Exp, scale=SEXP)
                                    if diag >= 0:
                                        dsl = slice(d0, d0 + P)
                                        tri_b = bass.AP(
                                            tensor=tri01[:].tensor,
                                            offset=tri01[:].offset,
                                            ap=[tri01[:].ap[0], [0, 2],
                                                *tri01[:].ap[1:]])
                                        nc.vector.tensor_tensor(
                                            Et[:, :, dsl], Et[:, :, dsl],
                                            tri_b, ALU.mult)
                                    nc.tensor.matmul(
                                        pa0[:, esl], lhsT=V[i][:, h0, :],
                                        rhs=Et[:, 0, esl],
                                        start=(i == 0), stop=(i == i_hi - 1))
                                    nc.tensor.matmul(
                                        pa1[:, esl], lhsT=V[i][:, h1, :],
                                        rhs=Et[:, 1, esl],
                                        start=(i == 0), stop=(i == i_hi - 1))
                                # denominators: recip(sums/8) = 8/sums
                                rr = rp.tile([Dh + 1, 2 * NF], F32, tag="rr")
                                nc.vector.reciprocal_approx_fast(
                                    out=rr[Dh:Dh + 1, 0:NF], in_=pa0[Dh:Dh + 1, :])
                                nc.vector.reciprocal_approx_fast(
                                    out=rr[Dh:Dh + 1, NF:2 * NF],
                                    in_=pa1[Dh:Dh + 1, :])
                                drs = dram.tile([2, NF], F32, tag="sums")
                                nc.sync.dma_start(out=drs[0:1, :],
                                                  in_=rr[Dh:Dh + 1, 0:NF])
                                nc.sync.dma_start(out=drs[1:2, :],
                                                  in_=rr[Dh:Dh + 1, NF:2 * NF])
                                bct = bp_.tile([Dh, 2, NF], F32, tag="bct")
                                nc.sync.dma_start(
                                    out=bct[:, 0, :],
                                    in_=drs[0:1, :].to_broadcast([Dh, NF]))
                                nc.sync.dma_start(
                                    out=bct[:, 1, :],
                                    in_=drs[1:2, :].to_broadcast([Dh, NF]))
                                nc.vector.tensor_tensor(
                                    atv[0:Dh, m, tsl], pa0[0:Dh, :],
                                    bct[:, 0, :], ALU.mult)
                                tmp1 = tp1.tile([Dh, NF], BF16, tag="t1")
                                nc.vector.tensor_tensor(
                                    tmp1[:], pa1[0:Dh, :], bct[:, 1, :], ALU.mult)
                                nc.gpsimd.dma_start(
                                    out=atv[Dh:2 * Dh, m, tsl], in_=tmp1[:])

                y_n = [arena.tile([P, C], BF16, tag=f"Y_{j}", name=f"y{j}")
                       for j in range(TJ)]
                if phases >= 4:
                    # ---------- Phase 3: proj + residual -> y (bf16, normal) -----
                    with ExitStack() as S:
                        xrb_p = S.enter_context(tc.tile_pool(name="xrbp", bufs=2))
                        for j in range(TJ):
                            xj2 = load_x(j)
                            xrb = xrb_p.tile([P, C], F32, tag="xrb")
                            nc.gpsimd.tensor_tensor(xrb[:], xj2[:], bpb[:], ALU.add)
                            for nn in range(TN):
                                csl = slice(nn * NF, (nn + 1) * NF)
                                pp = patile(nm="pp")
                                for k in range(KC):
                                    nc.tensor.matmul(
                                        pp[:], lhsT=atv[:, k, j * P:(j + 1) * P],
                                        rhs=wp_sb[k][:, csl],
                                        start=(k == 0), stop=(k == KC - 1))
                                nc.vector.tensor_tensor(
                                    y_n[j][:, csl], pp[:], xrb[:, csl], ALU.add)

                if phases >= 5:
                    # ---------------- Phase 4: LN2 ----------------
                    xn28 = mega("XN2")
                    ln_pass(lambda j: y_n[j], "xn2", xn28, 1)
                    xn2v = xn28[:].rearrange("p (k t) -> p k t", k=KC)

                hT = arena.tile([P, KC * T], BF16, tag="HT", name="hT")
                htv = hT[:].rearrange("p (m t) -> p m t", m=KC)
                if phases >= 6:
                    # ---------------- Phase 5: MLP fc1 + relu ----------------
                    for tn in range(TN):
                        tsl = slice(tn * NF, (tn + 1) * NF)
                        for m in range(KC):
                            ph = patile(nm="ph")
                            for q in range(QC):
                                nc.tensor.matmul(
                                    ph[:], lhsT=w1_sb[q][:, :, m * P:(m + 1) * P],
                                    rhs=xn2v[:, 2 * q:2 * q + 2, tsl],
                                    start=(q == 0), stop=(q == QC - 1),
                                    perf_mode=DR)
                            nc.scalar.activation(
                                out=htv[:, m, tsl], in_=ph[:], func=ACTF.Relu,
                                bias=cols["b1"][:, m:m + 1], scale=HS / WS)

                if phases >= 7:
                    # ---------- Phase 6: MLP fc2 + residual -> out ----------
                    with ExitStack() as S:
                        otp = S.enter_context(tc.tile_pool(name="otp", bufs=3))
                        y2p = S.enter_context(tc.tile_pool(name="y2p", bufs=2))
                        for j in range(TJ):
                            y2 = y2p.tile([P, C], BF16, tag="y2")
                            nc.gpsimd.tensor_tensor(y2[:], y_n[j][:], b2b[:],
                                                    ALU.add)
                            for nn in range(TN):
                                csl = slice(nn * NF, (nn + 1) * NF)
                                po = patile(nm="po")
                                for k in range(KC):
                                    nc.tensor.matmul(
                                        po[:], lhsT=htv[:, k, j * P:(j + 1) * P],
                                        rhs=w2_sb[k][:, csl],
                                        start=(k == 0), stop=(k == KC - 1))
                                ot = otp.tile([P, NF], F32, tag="ot")
                                nc.vector.scalar_tensor_tensor(
                                    out=ot[:], in0=po[:], scalar=1.0 / HS,
                                    in1=y2[:, csl], op0=ALU.mult, op1=ALU.add)
                                bulk_dma(out=out_d[j * P:(j + 1) * P, csl],
                                         in_=ot[:])

    nc.compile()
    return nc


def _f8(a):
    return np.clip(a, -240.0, 240.0).astype(F8NP)


def _pack_dr(w_eff):
    """[C, M] effective weight -> [QC*P, 2*M] fp8 DoubleRow layout
    (row q*128+p, col i*M+m  <-  w_eff[q*256 + i*128 + p, m] * WS)."""
    M = w_eff.shape[1]
    w = (w_eff * WS).reshape(QC, 2, P, M).transpose(0, 2, 1, 3).reshape(
        QC * P, 2 * M)
    return _f8(np.ascontiguousarray(w))


def _prep_inputs(inputs):
    """Host-side weight repacking/quantization; returns per-core in_maps."""
    f = np.float32
    x = np.ascontiguousarray(np.asarray(inputs["x"], dtype=f))        # [B, T, C]
    g1 = np.asarray(inputs["g1"], dtype=f)
    be1 = np.asarray(inputs["beta1"], dtype=f)
    g2 = np.asarray(inputs["g2"], dtype=f)
    be2 = np.asarray(inputs["beta2"], dtype=f)

    wq = np.asarray(inputs["Wq"], dtype=f).transpose(1, 0, 2).reshape(C, C)
    wk = np.asarray(inputs["Wk"], dtype=f).transpose(1, 0, 2).reshape(C, C)
    wv = np.asarray(inputs["Wv"], dtype=f).transpose(1, 0, 2).reshape(C, C)
    wp = np.asarray(inputs["Wproj"], dtype=f)
    w1 = np.asarray(inputs["W1"], dtype=f)
    w2 = np.asarray(inputs["W2"], dtype=f)
    b1 = np.asarray(inputs["b1"], dtype=f)

    common = {
        "wq": _pack_dr(g1[:, None] * wq),
        "wk": _pack_dr(g1[:, None] * wk),
        "wv": _pack_dr(g1[:, None] * wv),
        "wp": np.ascontiguousarray(wp).astype(BF16NP),
        "w1": _pack_dr(g2[:, None] * w1),
        "w2": np.ascontiguousarray(w2).astype(BF16NP),
        "colq": QS * (be1 @ wq),
        "colk": QS * (be1 @ wk),
        "b1c": HS * (b1 + be2 @ w1),
        "bvr": (WS * (be1 @ wv)).astype(BF16NP),
        "bp": np.asarray(inputs["bproj"], dtype=f),
        "b2": np.asarray(inputs["b2"], dtype=f),
    }
    return [{"x": x[b], **common} for b in range(N_CORES)]


def kernel(**inputs) -> np.ndarray:
    if "nc" not in _CACHE:
        _CACHE["nc"] = build_nc()
    nc = _CACHE["nc"]
    in_maps = _prep_inputs(inputs)
    res = run_bass_kernel_spmd(nc, in_maps, list(range(N_CORES)))
    out = np.stack([res.results[b]["out"] for b in range(N_CORES)], axis=0)
    return out.astype(np.float32)


if __name__ == "__main__":
    rng = np.random.default_rng(0)
    demo = {
        "x": rng.standard_normal((B, T, C), dtype=np.float32),
        "Wq": rng.standard_normal((H, C, Dh), dtype=np.float32) * 0.02,
        "Wk": rng.standard_normal((H, C, Dh), dtype=np.float32) * 0.02,
        "Wv": rng.standard_normal((H, C, Dh), dtype=np.float32) * 0.02,
        "Wproj": rng.standard_normal((C, C), dtype=np.float32) * 0.02,
        "bproj": np.zeros(C, np.float32),
        "W1": rng.standard_normal((C, C), dtype=np.float32) * 0.02,
        "b1": np.zeros(C, np.float32),
        "W2": rng.standard_normal((C, C), dtype=np.float32) * 0.02,
        "b2": np.zeros(C, np.float32),
        "g1": np.ones(C, np.float32),
        "beta1": np.zeros(C, np.float32),
        "g2": np.ones(C, np.float32),
        "beta2": np.zeros(C, np.float32),
    }
    y = kernel(**demo)
    print("out", y.shape, y.dtype, float(np.abs(y).max()))


# revision 35
# speedup vs baseline: 1.8294x; 1.8162x over previous
"""Trainium2 Bass kernel for nn_Block_3616362463321 (dense transformer block).

B=8, T=1024, C=1024, H=16, Dh=64. Data-parallel over batch: core b gets x[b].
Weights replicated to all 8 cores; no collectives.

v2 design (vs baseline):
  - All six weight GEMMs (QKV / proj / fc1 / fc2) run fp8e4m3 with
    perf_mode=DoubleRow: K=256 contraction per pass, 0.5 cycles/row.
    Weights are host-quantized (x1024, clip +-240) into an interleaved
    [q, p, islot, m] layout; activations are quantized on the fly into
    "mega" SBUF tiles [128, 8*1024] fp8 whose (k=c-chunk, t) layout serves
    both the DoubleRow moving-operand view [p, 2, t] and the stationary
    view [p, 2, 128].
  - LayerNorm statistics AND normalization happen in natural [t, c] layout
    (per-partition mean/rstd -> one tensor_scalar), with gamma folded into
    the weights host-side and beta folded into per-output bias columns.
    No DRAM stat bounces.  Normalized activations are PE-transposed as fp8
    (1 cycle/row), 4 blocks batched per PSUM bank.
  - Attention keeps the baseline S^T orientation (fp8 Q/K, bf16 E,
    fp8 V with fused 0.125-ones column for softmax denominators), with:
      * S row-tile pairs (K=64 at rows 0/64) writing one 2-bank PSUM pair
        tile -> a single paired exp per (m, tn, i) on ACT,
      * exact causal trims everywhere (bf16/fp8 matmuls have no N>=256
        restriction),
      * causal masking as a post-exp bf16 triangle multiply (2x DVE mode),
      * reciprocal_approx_fast for denominators + SBUF->SBUF broadcast DMA,
  - Residual/dequant fused into single scalar_tensor_tensor ops; SBUF-only
    elementwise work (x+bias rows) offloaded to the idle GpSimd engine.
  - Bulk DMA spread over the SP / Pool / ACT hardware queues.
"""
import sys

sys.path.insert(0, "/opt/trn_rl_repo")

from contextlib import ExitStack, nullcontext

import numpy as np
import ml_dtypes

import concourse.bacc as bacc
import concourse.bass as bass
import concourse.mybir as mybir
import concourse.tile as tile
from concourse.bass_utils import run_bass_kernel_spmd

P = 128
B, T, C, H = 8, 1024, 1024, 16
Dh = C // H            # 64
EPS = 1e-5
NF = 512               # matmul moving free dim (fp32 PSUM bank limit)
KC = C // P            # 8 c-chunks of 128
QC = C // 256          # 4 c-chunks of 256 (DoubleRow)
TJ = T // P            # 8 t-chunks of 128
TN = T // NF           # 2 t-chunks of 512
F32 = mybir.dt.float32
F32R = mybir.dt.float32r
BF16 = mybir.dt.bfloat16
F8 = mybir.dt.float8e4
ALU = mybir.AluOpType
ACTF = mybir.ActivationFunctionType
DR = mybir.MatmulPerfMode.DoubleRow

WS = 1024.0            # host weight upscale (fp8 range use)
QS = 8.0               # Q/K storage scale
SEXP = (Dh ** -0.5) / (QS * QS)   # exp scale absorbing Q/K storage scales
VIS = 1.0              # V ones-column value (bf16 attnT: true scale)
HS = 8.0               # hT storage scale

N_CORES = 8

_CACHE = {}

F8NP = ml_dtypes.float8_e4m3
BF16NP = ml_dtypes.bfloat16


def _bcast_row_ap(handle_ap, parts):
    """AP reading a [N]-shaped DRAM tensor broadcast across `parts` partitions."""
    return bass.AP(
        tensor=handle_ap.tensor,
        offset=handle_ap.offset,
        ap=[[0, parts], *handle_ap.ap],
    )


def build_nc(loop=1, hwloop=0, phases=7):
    nc = bacc.Bacc("TRN2", target_bir_lowering=False, debug=False)

    x_d = nc.dram_tensor("x", [T, C], F32, kind="ExternalInput")
    w_d = {}
    for nm in ("wq", "wk", "wv", "w1"):
        w_d[nm] = nc.dram_tensor(nm, [QC * P, 2 * C], F8, kind="ExternalInput")
    wp_d = nc.dram_tensor("wp", [C, C], BF16, kind="ExternalInput")
    w2_d = nc.dram_tensor("w2", [C, C], BF16, kind="ExternalInput")
    colq_d = nc.dram_tensor("colq", [C], F32, kind="ExternalInput")
    colk_d = nc.dram_tensor("colk", [C], F32, kind="ExternalInput")
    b1c_d = nc.dram_tensor("b1c", [C], F32, kind="ExternalInput")
    bvr_d = nc.dram_tensor("bvr", [C], BF16, kind="ExternalInput")
    bp_d = nc.dram_tensor("bp", [C], F32, kind="ExternalInput")
    b2_d = nc.dram_tensor("b2", [C], F32, kind="ExternalInput")
    out_d = nc.dram_tensor("out", [T, C], F32, kind="ExternalOutput")

    identb_c = nc.inline_tensor(np.eye(P).astype(BF16NP), name="identb_c")
    # post-exp causal keep-mask for a diagonal [s,t] block: keep where s <= t
    tri_np = (np.arange(P)[:, None] <= np.arange(P)[None, :]).astype(BF16NP)
    tri_c = nc.inline_tensor(tri_np, name="tri_c")

    with tile.TileContext(nc) as tc, ExitStack() as ES:
        singles = ES.enter_context(tc.tile_pool(name="singles", bufs=1))
        dram = ES.enter_context(tc.tile_pool(name="drsc", bufs=1, space="DRAM"))

        identb = singles.tile([P, P], BF16)
        nc.sync.dma_start(out=identb[:], in_=identb_c.ap())
        tri01 = singles.tile([P, P], BF16)
        nc.sync.dma_start(out=tri01[:], in_=tri_c.ap())
        epsc = singles.tile([P, 1], F32)
        nc.vector.memset(epsc[:], EPS)
        onesb = singles.tile([1, P], BF16)
        nc.vector.memset(onesb[:], 1.0)

        # bias columns [P, KC]: element (p, k) = vec[k*P + p]
        cols = {}

        def load_col(nm, hd):
            t_ = singles.tile([P, KC], F32, tag=f"col_{nm}", name=f"col_{nm}")
            nc.sync.dma_start(out=t_[:], in_=hd.ap().rearrange("(k p) -> p k", p=P))
            cols[nm] = t_

        load_col("q", colq_d)
        load_col("k", colk_d)
        load_col("b1", b1c_d)
        bvr = singles.tile([1, C], BF16)
        nc.gpsimd.dma_start(out=bvr[:], in_=bvr_d.ap()[None, :])
        bpb = singles.tile([P, C], F32)
        nc.gpsimd.dma_start(out=bpb[:], in_=_bcast_row_ap(bp_d.ap(), P))
        b2b = singles.tile([P, C], F32)
        nc.gpsimd.dma_start(out=b2b[:], in_=_bcast_row_ap(b2_d.ap(), P))

        # ---- SBUF arena ----
        arena = ES.enter_context(tc.tile_pool(name="arena", bufs=1))

        def mega(tag):
            return arena.tile([P, KC * T], F8, tag=tag, name=tag)

        # weight tiles: [P, 2, C] fp8 per 256-chunk
        def wtiles(nm, share=None):
            tg = share or nm
            return [arena.tile([P, 2, C], F8, tag=f"{tg}_{q}", name=f"{nm}{q}")
                    for q in range(QC)]

        # ---- PSUM pool: declare SP pair tags first (2 banks each), then PA ----
        psum = ES.enter_context(tc.tile_pool(name="psum", bufs=1, space="PSUM"))
        _sp = [0]
        _pa = [0]

        def sptile(shape, dtype, nm="sp"):
            t = psum.tile(list(shape), dtype, tag=f"SP{_sp[0] % 2}",
                          name=f"{nm}{_sp[0]}")
            _sp[0] += 1
            return t

        def patile(shape=(P, NF), nm="pa"):
            t = psum.tile(list(shape), F32, tag=f"PA{_pa[0] % 4}",
                          name=f"{nm}{_pa[0]}")
            _pa[0] += 1
            return t

        # force tag declaration order: SP0, SP1 as [P, 2, NF] f32 (2 banks each)
        _ = psum.tile([P, 2, NF], F32, tag="SP0", name="spdecl0")
        _ = psum.tile([P, 2, NF], F32, tag="SP1", name="spdecl1")

        # weights on the Pool/ACT queues; x and out rows keep SP (+ACT) free-ish
        _dq = [0]
        _dqe = [nc.gpsimd, nc.scalar]

        def bulk_dma(out, in_):
            eng = _dqe[_dq[0] % len(_dqe)]
            _dq[0] += 1
            eng.dma_start(out=out, in_=in_)

        def row_dma(out, in_):
            nc.sync.dma_start(out=out, in_=in_)

        def ln_pass(src_tiles_or_loader, xn_tag, dst_mega, ph, out_rows_dtype=F8):
            """Natural-layout LN: per 128-row chunk j, bn_stats -> mean/rstd
            columns -> one tensor_scalar into an fp8 row tile -> PE-transpose
            (fp8, 4 blocks per PSUM batch) into dst_mega [(k t)] layout."""
            with ExitStack() as S:
                stp = S.enter_context(tc.tile_pool(name=f"stp{ph}", bufs=4))
                xnp = S.enter_context(tc.tile_pool(name=f"xnp{ph}", bufs=3))
                dv = dst_mega[:].rearrange("p (k t) -> p k t", k=KC)
                for j in range(TJ):
                    xj = src_tiles_or_loader(j)
                    st = stp.tile([P, 2, 6], F32, tag="st")
                    xr2 = xj[:].rearrange("p (g f) -> p g f", f=NF)
                    for g in range(2):
                        nc.vector.bn_stats(out=st[:, g, :], in_=xr2[:, g, :])
                    mv = stp.tile([P, 2], F32, tag="mv")
                    nc.vector.bn_aggr(out=mv[:], in_=st[:])
                    srt = stp.tile([P, 1], F32, tag="srt")
                    nc.scalar.activation(out=srt[:], in_=mv[:, 1:2],
                                         func=ACTF.Sqrt, bias=epsc[:], scale=1.0)
                    rc = stp.tile([P, 1], F32, tag="rc")
                    nc.vector.reciprocal(rc[:], srt[:])
                    xnr = xnp.tile([P, C], BF16, tag="xnr")
                    nc.vector.tensor_scalar(
                        out=xnr[:], in0=xj[:], scalar1=mv[:, 0:1], scalar2=rc[:],
                        op0=ALU.subtract, op1=ALU.mult)
                    for kb in range(2):
                        pt = sptile([P, 4 * P], BF16, "pt")
                        for k4 in range(4):
                            k = kb * 4 + k4
                            nc.tensor.transpose(pt[:, k4 * P:(k4 + 1) * P],
                                                xnr[:, k * P:(k + 1) * P],
                                                identb[:])
                        dst = dv[:, kb * 4:(kb + 1) * 4, j * P:(j + 1) * P]
                        src = pt[:].rearrange("p (a b) -> p a b", a=4)
                        if kb == 0:
                            nc.vector.tensor_copy(out=dst, in_=src)
                        else:
                            nc.scalar.activation(out=dst, in_=src,
                                                 func=ACTF.Copy)

        xrp = ES.enter_context(tc.tile_pool(name="xrp", bufs=3))

        def load_x(j):
            xj = xrp.tile([P, C], F32, tag="xrow")
            row_dma(out=xj[:], in_=x_d[j * P:(j + 1) * P, :])
            return xj

        with (tc.For_i(0, hwloop, 1) if hwloop else nullcontext()):
            for _it in range(loop):
                # ---------- weights: issue all DMAs up front (prefetch) ----------
                wq_sb = wtiles("wq")
                wk_sb = wtiles("wk")
                wv_sb = wtiles("wv")
                # w1/w2 reuse wv/wq slots (dead after QKV); DMAs self-order
                w1_sb = wtiles("w1", share="wv")
                w2_sb = [arena.tile([P, C], BF16,
                                    tag=(f"wq_{k}" if k < QC else f"wk_{k - QC}"),
                                    name=f"w2{k}") for k in range(KC)]
                for q in range(QC):
                    for nm, tl in (("wq", wq_sb), ("wk", wk_sb), ("wv", wv_sb)):
                        bulk_dma(out=tl[q][:], in_=w_d[nm][q * P:(q + 1) * P, :])
                # proj weights: plain bf16 [P, C] per c'-chunk
                wp_sb = [arena.tile([P, C], BF16, tag=f"wp_{k}", name=f"wp{k}")
                         for k in range(KC)]
                for k in range(KC):
                    bulk_dma(out=wp_sb[k][:], in_=wp_d[k * P:(k + 1) * P, :])

                # ---------- Phase 0: LN1 (stats + normalize + transpose) ----------
                xn8 = mega("XN1")
                ln_pass(load_x, "xn", xn8, 0)
                xnv = xn8[:].rearrange("p (k t) -> p k t", k=KC)

                if phases >= 2:
                    # ---------------- Phase 1: QKV (fp8 DoubleRow) ----------------
                    QT = [arena.tile([P, T], F8, tag=f"QT_{m}", name=f"QT{m}")
                          for m in range(KC)]
                    KTt = [arena.tile([P, T], F8, tag=f"KT_{m}", name=f"KT{m}")
                           for m in range(KC)]
                    for tn in range(TN):
                        tsl = slice(tn * NF, (tn + 1) * NF)
                        for m in range(KC):
                            pq = patile(nm="pq")
                            for q in range(QC):
                                nc.tensor.matmul(
                                    pq[:], lhsT=wq_sb[q][:, :, m * P:(m + 1) * P],
                                    rhs=xnv[:, 2 * q:2 * q + 2, tsl],
                                    start=(q == 0), stop=(q == QC - 1),
                                    perf_mode=DR)
                            nc.vector.tensor_scalar(
                                out=QT[m][:, tsl], in0=pq[:], scalar1=QS / WS,
                                scalar2=cols["q"][:, m:m + 1],
                                op0=ALU.mult, op1=ALU.add)
                            pk = patile(nm="pk")
                            for q in range(QC):
                                nc.tensor.matmul(
                                    pk[:], lhsT=wk_sb[q][:, :, m * P:(m + 1) * P],
                                    rhs=xnv[:, 2 * q:2 * q + 2, tsl],
                                    start=(q == 0), stop=(q == QC - 1),
                                    perf_mode=DR)
                            nc.scalar.activation(
                                out=KTt[m][:, tsl], in_=pk[:], func=ACTF.Identity,
                                scale=QS / WS, bias=cols["k"][:, m:m + 1])

                    V = [arena.tile([P, H, Dh + 1], BF16, tag=f"V_{j}",
                                    name=f"V{j}") for j in range(TJ)]
                    for j in range(TJ):
                        nc.gpsimd.memset(V[j][:, :, Dh:Dh + 1], VIS)
                        for hn in range(TN):
                            hsl = slice(hn * NF, (hn + 1) * NF)
                            pv = patile(nm="pv")
                            nc.tensor.matmul(pv[:], lhsT=onesb[0:1, :],
                                             rhs=bvr[0:1, hsl],
                                             start=True, stop=False)
                            for q in range(QC):
                                nc.tensor.matmul(
                                    pv[:], lhsT=xnv[:, 2 * q:2 * q + 2,
                                                    j * P:(j + 1) * P],
                                    rhs=wv_sb[q][:, :, hsl],
                                    start=False, stop=(q == QC - 1),
                                    perf_mode=DR)
                            nc.scalar.activation(
                                out=V[j][:, hn * 8:(hn + 1) * 8, 0:Dh],
                                in_=pv[:].rearrange("p (h d) -> p h d", d=Dh),
                                func=ACTF.Identity, scale=1.0 / WS)
                    # late weights into the now-free wv/wq/wk slots
                    for q in range(QC):
                        bulk_dma(out=w1_sb[q][:],
                                 in_=w_d["w1"][q * P:(q + 1) * P, :])
                    for k in range(KC):
                        bulk_dma(out=w2_sb[k][:],
                                 in_=w2_d[k * P:(k + 1) * P, :])

                # proj residual rows (x + bproj) prefetched before attention
                # so proj can start the moment its attnT half is ready
                xrb = [arena.tile([P, C], BF16, tag=f"XRB_{j}", name=f"xrb{j}")
                       for j in range(TJ)]
                for j in range(TJ):
                    xj2 = load_x(j)
                    nc.gpsimd.tensor_tensor(xrb[j][:], xj2[:], bpb[:], ALU.add)

                attnT = arena.tile([P, KC * T], BF16, tag="ATT", name="attnT")
                atv = attnT[:].rearrange("p (m t) -> p m t", m=KC)
                if phases >= 3:
                    # ---------------- Phase 2: attention ----------------
                    with ExitStack() as S:
                        ep = S.enter_context(tc.tile_pool(name="ep", bufs=3))
                        rp = S.enter_context(tc.tile_pool(name="rp", bufs=3))
                        bp_ = S.enter_context(tc.tile_pool(name="bp", bufs=2))
                        tp1 = S.enter_context(tc.tile_pool(name="tp1", bufs=2))
                        for tn in range(TN):
                            tsl = slice(tn * NF, (tn + 1) * NF)
                            i_hi = 4 * (tn + 1)
                            for m in range(KC):
                                h0, h1 = 2 * m, 2 * m + 1
                                pa0 = patile((Dh + 1, NF), "pa0")
                                pa1 = patile((Dh + 1, NF), "pa1")
                                for i in range(i_hi):
                                    diag = i - 4 * tn
                                    d0 = max(diag, 0) * P
                                    esl = slice(d0, NF)
                                    qsl = slice(tn * NF + d0, (tn + 1) * NF)
                                    ssl = slice(i * P, (i + 1) * P)
                                    sp2 = sptile([P, 2, NF], F32, "s")
                                    nc.tensor.matmul(
                                        sp2[:, 0, esl], lhsT=KTt[m][0:64, ssl],
                                        rhs=QT[m][0:64, qsl],
                                        start=True, stop=True)
                                    nc.tensor.matmul(
                                        sp2[:, 1, esl], lhsT=KTt[m][64:128, ssl],
                                        rhs=QT[m][64:128, qsl],
                                        start=True, stop=True)
                                    Et = ep.tile([P, 2, NF], BF16, tag="E")
                                    nc.scalar.activation(
                                        out=Et[:, :, esl], in_=sp2[:, :, esl],
                                        func=ACTF.Exp, scale=SEXP)
                                    if diag >= 0:
                                        dsl = slice(d0, d0 + P)
                                        tri_b = bass.AP(
                                            tensor=tri01[:].tensor,
                                            offset=tri01[:].offset,
                                            ap=[tri01[:].ap[0], [0, 2],
                                                *tri01[:].ap[1:]])
                                        nc.vector.tensor_tensor(
                                            Et[:, :, dsl], Et[:, :, dsl],
                                            tri_b, ALU.mult)
                                    nc.tensor.matmul(
                                        pa0[:, esl], lhsT=V[i][:, h0, :],
                                        rhs=Et[:, 0, esl],
                                        start=(i == 0), stop=(i == i_hi - 1))
                                    nc.tensor.matmul(
                                        pa1[:, esl], lhsT=V[i][:, h1, :],
                                        rhs=Et[:, 1, esl],
                                        start=(i == 0), stop=(i == i_hi - 1))
                                # evacuate pa to SBUF immediately (frees the
                                # PSUM banks from the denominator-bounce
                                # latency), then: reshape sums to [128, 8] via
                                # DMA so the bit-exact reciprocal runs wide,
                                # bounce through DRAM for the broadcast, and
                                # normalize on the idle Pool engine.
                                av0 = rp.tile([Dh + 1, NF], F32, tag="av0")
                                av1 = rp.tile([Dh + 1, NF], F32, tag="av1")
                                nc.vector.tensor_copy(out=av0[:], in_=pa0[:])
                                nc.vector.tensor_copy(out=av1[:], in_=pa1[:])
                                s2 = rp.tile([P, 8], F32, tag="s2")
                                nc.gpsimd.dma_start(
                                    out=s2[:, 0:4],
                                    in_=av0[Dh:Dh + 1, :])
                                nc.gpsimd.dma_start(
                                    out=s2[:, 4:8],
                                    in_=av1[Dh:Dh + 1, :])
                                nc.vector.reciprocal(s2[:], s2[:])
                                drr = dram.tile([2 * NF], F32, tag="rsums")
                                nc.gpsimd.dma_start(
                                    out=drr[0:NF].rearrange("(p i) -> p i", i=4),
                                    in_=s2[:, 0:4])
                                nc.gpsimd.dma_start(
                                    out=drr[NF:2 * NF].rearrange(
                                        "(p i) -> p i", i=4),
                                    in_=s2[:, 4:8])
                                bct = bp_.tile([Dh, 2, NF], F32, tag="bct")
                                nc.sync.dma_start(
                                    out=bct[:, 0, :],
                                    in_=drr[0:NF][None, :].to_broadcast([Dh, NF]))
                                nc.sync.dma_start(
                                    out=bct[:, 1, :],
                                    in_=drr[NF:2 * NF][None, :].to_broadcast(
                                        [Dh, NF]))
                                nc.gpsimd.tensor_tensor(
                                    atv[0:Dh, m, tsl], av0[0:Dh, :],
                                    bct[:, 0, :], ALU.mult)
                                tmp1 = tp1.tile([Dh, NF], BF16, tag="t1")
                                nc.gpsimd.tensor_tensor(
                                    tmp1[:], av1[0:Dh, :], bct[:, 1, :], ALU.mult)
                                nc.gpsimd.dma_start(
                                    out=atv[Dh:2 * Dh, m, tsl], in_=tmp1[:])

                y_n = [arena.tile([P, C], BF16, tag=f"Y_{j}", name=f"y{j}")
                       for j in range(TJ)]
                if phases >= 4:
                    # ---------- Phase 3: proj + residual -> y (bf16, normal) -----
                    for j in range(TJ):
                        for nn in range(TN):
                            csl = slice(nn * NF, (nn + 1) * NF)
                            pp = patile(nm="pp")
                            for k in range(KC):
                                nc.tensor.matmul(
                                    pp[:], lhsT=atv[:, k, j * P:(j + 1) * P],
                                    rhs=wp_sb[k][:, csl],
                                    start=(k == 0), stop=(k == KC - 1))
                            nc.vector.tensor_tensor(
                                y_n[j][:, csl], pp[:], xrb[j][:, csl], ALU.add)

                if phases >= 5:
                    # ---------------- Phase 4: LN2 ----------------
                    xn28 = mega("XN2")
                    ln_pass(lambda j: y_n[j], "xn2", xn28, 1)
                    xn2v = xn28[:].rearrange("p (k t) -> p k t", k=KC)

                hT = arena.tile([P, KC * T], BF16, tag="HT", name="hT")
                htv = hT[:].rearrange("p (m t) -> p m t", m=KC)
                if phases >= 6:
                    # ---------------- Phase 5: MLP fc1 + relu ----------------
                    for tn in range(TN):
                        tsl = slice(tn * NF, (tn + 1) * NF)
                        for m in range(KC):
                            ph = patile(nm="ph")
                            for q in range(QC):
                                nc.tensor.matmul(
                                    ph[:], lhsT=w1_sb[q][:, :, m * P:(m + 1) * P],
                                    rhs=xn2v[:, 2 * q:2 * q + 2, tsl],
                                    start=(q == 0), stop=(q == QC - 1),
                                    perf_mode=DR)
                            nc.scalar.activation(
                                out=htv[:, m, tsl], in_=ph[:], func=ACTF.Relu,
                                bias=cols["b1"][:, m:m + 1], scale=HS / WS)

                if phases >= 7:
                    # ---------- Phase 6: MLP fc2 + residual -> out ----------
                    with ExitStack() as S:
                        otp = S.enter_context(tc.tile_pool(name="otp", bufs=3))
                        y2p = S.enter_context(tc.tile_pool(name="y2p", bufs=2))
                        for j in range(TJ):
                            y2 = y2p.tile([P, C], BF16, tag="y2")
                            nc.gpsimd.tensor_tensor(y2[:], y_n[j][:], b2b[:],
                                                    ALU.add)
                            for nn in range(TN):
                                csl = slice(nn * NF, (nn + 1) * NF)
                                po = patile(nm="po")
                                for k in range(KC):
                                    nc.tensor.matmul(
                                        po[:], lhsT=htv[:, k, j * P:(j + 1) * P],
                                        rhs=w2_sb[k][:, csl],
                                        start=(k == 0), stop=(k == KC - 1))
                                ot = otp.tile([P, NF], F32, tag="ot")
                                nc.vector.scalar_tensor_tensor(
                                    out=ot[:], in0=po[:], scalar=1.0 / HS,
                                    in1=y2[:, csl], op0=ALU.mult, op1=ALU.add)
                                bulk_dma(out=out_d[j * P:(j + 1) * P, csl],
                                         in_=ot[:])

    nc.compile()
    return nc


def _f8(a):
    return np.clip(a, -240.0, 240.0).astype(F8NP)


def _pack_dr(w_eff):
    """[C, M] effective weight -> [QC*P, 2*M] fp8 DoubleRow layout
    (row q*128+p, col i*M+m  <-  w_eff[q*256 + i*128 + p, m] * WS)."""
    M = w_eff.shape[1]
    w = (w_eff * WS).reshape(QC, 2, P, M).transpose(0, 2, 1, 3).reshape(
        QC * P, 2 * M)
    return _f8(np.ascontiguousarray(w))


def _prep_inputs(inputs):
    """Host-side weight repacking/quantization; returns per-core in_maps."""
    f = np.float32
    x = np.ascontiguousarray(np.asarray(inputs["x"], dtype=f))        # [B, T, C]
    g1 = np.asarray(inputs["g1"], dtype=f)
    be1 = np.asarray(inputs["beta1"], dtype=f)
    g2 = np.asarray(inputs["g2"], dtype=f)
    be2 = np.asarray(inputs["beta2"], dtype=f)

    wq = np.asarray(inputs["Wq"], dtype=f).transpose(1, 0, 2).reshape(C, C)
    wk = np.asarray(inputs["Wk"], dtype=f).transpose(1, 0, 2).reshape(C, C)
    wv = np.asarray(inputs["Wv"], dtype=f).transpose(1, 0, 2).reshape(C, C)
    wp = np.asarray(inputs["Wproj"], dtype=f)
    w1 = np.asarray(inputs["W1"], dtype=f)
    w2 = np.asarray(inputs["W2"], dtype=f)
    b1 = np.asarray(inputs["b1"], dtype=f)

    common = {
        "wq": _pack_dr(g1[:, None] * wq),
        "wk": _pack_dr(g1[:, None] * wk),
        "wv": _pack_dr(g1[:, None] * wv),
        "wp": np.ascontiguousarray(wp).astype(BF16NP),
        "w1": _pack_dr(g2[:, None] * w1),
        "w2": np.ascontiguousarray(w2).astype(BF16NP),
        "colq": QS * (be1 @ wq),
        "colk": QS * (be1 @ wk),
        "b1c": HS * (b1 + be2 @ w1),
        "bvr": (WS * (be1 @ wv)).astype(BF16NP),
        "bp": np.asarray(inputs["bproj"], dtype=f),
        "b2": np.asarray(inputs["b2"], dtype=f),
    }
    return [{"x": x[b], **common} for b in range(N_CORES)]


def kernel(**inputs) -> np.ndarray:
    if "nc" not in _CACHE:
        _CACHE["nc"] = build_nc()
    nc = _CACHE["nc"]
    in_maps = _prep_inputs(inputs)
    res = run_bass_kernel_spmd(nc, in_maps, list(range(N_CORES)))
    out = np.stack([res.results[b]["out"] for b in range(N_CORES)], axis=0)
    return out.astype(np.float32)


if __name__ == "__main__":
    rng = np.random.default_rng(0)
    demo = {
        "x": rng.standard_normal((B, T, C), dtype=np.float32),
        "Wq": rng.standard_normal((H, C, Dh), dtype=np.float32) * 0.02,
        "Wk": rng.standard_normal((H, C, Dh), dtype=np.float32) * 0.02,
        "Wv": rng.standard_normal((H, C, Dh), dtype=np.float32) * 0.02,
        "Wproj": rng.standard_normal((C, C), dtype=np.float32) * 0.02,
        "bproj": np.zeros(C, np.float32),
        "W1": rng.standard_normal((C, C), dtype=np.float32) * 0.02,
        "b1": np.zeros(C, np.float32),
        "W2": rng.standard_normal((C, C), dtype=np.float32) * 0.02,
        "b2": np.zeros(C, np.float32),
        "g1": np.ones(C, np.float32),
        "beta1": np.zeros(C, np.float32),
        "g2": np.ones(C, np.float32),
        "beta2": np.zeros(C, np.float32),
    }
    y = kernel(**demo)
    print("out", y.shape, y.dtype, float(np.abs(y).max()))
